# revision 16
# baseline (speedup 1.0000x reference)
"""VQ codebook bottleneck kernel for Trainium2, data-parallel over 8 NeuronCores.

Problem: x (8, 64, 8192) f32, codebook k (2048, 64) f32.
Per batch row n (= one core): tokens are columns of x[n] (64, 8192).
  dist[t, j] = |x_t|^2 - 2 x_t.k_j + |k_j|^2
  labels = argmin_j dist, x_d = k[labels] (straight-through == plain gather
  up to fp rounding), fit = mean(min dist), commit = fit/64,
  prenorm = sqrt(mean(x^2) - mean(x)^2).

Device algorithm per core (8192 tokens, 64 token-tiles of 128):
  s[t, j] = x_t.k_j - |k_j|^2/2 - |x_t|^2/2 = -dist[t, j]/2
  The contraction carries 66 rows: 64 x-dims, a ones row paired with the
  -|k|^2/2 rhs row, and a -|x|^2/2 row paired with a ones rhs row.
  Matmuls run as an fp16 three-term split (hi*hi + hi*lo + lo*hi, fp32
  accumulate): ~4e-5 absolute error (vs 1e-5 for native fp32) at ~2x the
  throughput, because fp32 matmuls self-load their weights serially while
  fp16 LDWEIGHTS pipelines through the PE reorder window.
  argmin dist == argmax s;  min dist = -2 max_j s.  VectorE max/max_index
  give the exact argmax (first occurrence on ties, like jnp.argmin).
  GPSIMD indirect DMA gathers codebook rows for x_d.  Scalar partials are
  reduced on device; the 8 per-core partials are combined on host (the
  unshard step).
"""
import numpy as np

N, W, T = 8, 64, 8192
KB = 2048
NT = N * T
TILE = 128
NTILES = T // TILE  # 64
CH = 512
KDIM = W + 2        # 66 contraction rows of the main matmul
XG = 1024           # x2-prep group width
NXG = T // XG       # 8

_CACHE = {}


def _build(repeat: int = 1, parts: str = "mm,max,idx,smalls,gat"):
    parts = set(parts.split(","))
    import contextlib
    import concourse.bacc as bacc
    import concourse.bass as bass
    import concourse.bass_isa as bass_isa
    import concourse.mybir as mybir
    from concourse.tile import TileContext

    f32 = mybir.dt.float32
    f16 = mybir.dt.float16
    u32 = mybir.dt.uint32
    fp32_mm = "mmfp32" in parts
    assert not fp32_mm, "fp32 matmul fallback removed in the 2-matmul split"

    nc = bacc.Bacc(None, target_bir_lowering=False)
    xw_d = nc.dram_tensor("xw", [W, T], f32, kind="ExternalInput")
    kt_d = nc.dram_tensor("kt", [W, KB], f32, kind="ExternalInput")
    ktab_d = nc.dram_tensor("ktab", [KB, W], f32, kind="ExternalInput")
    xd_d = nc.dram_tensor("xd", [T, W], f32, kind="ExternalOutput")
    lab_d = nc.dram_tensor("lab", [TILE, NTILES], u32, kind="ExternalOutput")
    par_d = nc.dram_tensor("par", [TILE, 4], f32, kind="ExternalOutput")
    par2_d = nc.dram_tensor("par2", [1, 2 * NXG], f32, kind="ExternalOutput")

    with TileContext(nc) as tc:
        with tc.tile_pool(name="big", bufs=1) as big, \
             tc.tile_pool(name="ps_s", bufs=2, space="PSUM") as ps_s, \
             tc.tile_pool(name="work", bufs=3) as work, \
             tc.tile_pool(name="x2w", bufs=2) as x2w, \
             tc.tile_pool(name="gatp", bufs=4) as gatp:

            # ---- static tensors ----
            xbig = big.tile([W, T], f32)      # plain x
            nc.sync.dma_start(xbig[:], xw_d[:])

            # r rows: 0..63 = kT, 64 = ones, 65 = -|k|^2/2 (fp32 reference
            # copy, used by the fp32 fallback and to derive fp16 operands)
            r_t = big.tile([KDIM, KB], f32)
            nc.sync.dma_start(r_t[0:W, :], kt_d[:])
            nc.vector.memset(r_t[W:W + 2, :], 1.0)

            hk_t = big.tile([W, KB], f32)     # kT*kT elementwise
            nc.vector.tensor_mul(hk_t[:], r_t[0:W, :], r_t[0:W, :])

            k2bc = big.tile([W, KB], f32)
            nc.gpsimd.partition_all_reduce(k2bc[:], hk_t[:], W,
                                           bass_isa.ReduceOp.add)
            k2scr = big.tile([1, KB], f32)
            nc.scalar.mul(k2scr[0:1, :], k2bc[0:1, :], -0.5)
            nc.sync.dma_start(r_t[W + 1:W + 2, :], k2scr[0:1, :])

            if not fp32_mm:
                # fp16 operands for the two-matmul split per chunk:
                #   main  (K=66):  xm=[hi(x); 1; 1]  .  rm=[hi(kT); bias_hi; bias_lo]
                #   cross (K=128): xc=[hi(x); lo(x)] . rc=[lo(kT); hi(kT)]
                # bias = -|k|^2/2.  Rows 64/65 of rm are staged on partitions
                # 0/1 and DMA'd in (engine APs start at partition 0/32/64/96).
                rm = big.tile([KDIM, KB], f16)
                nc.scalar.copy(rm[0:W, :], r_t[0:W, :])
                rc = big.tile([2 * W, KB], f16)
                nc.scalar.copy(rc[W:2 * W, :], r_t[0:W, :])
                rl32 = big.tile([W, KB], f32)
                nc.vector.tensor_sub(rl32[:], r_t[0:W, :], rm[0:W, :])
                nc.scalar.copy(rc[0:W, :], rl32[:])
                bhi = big.tile([1, KB], f16)
                nc.scalar.copy(bhi[0:1, :], k2scr[0:1, :])
                blo32 = big.tile([1, KB], f32)
                nc.vector.tensor_sub(blo32[:], k2scr[0:1, :], bhi[0:1, :])
                blo = big.tile([1, KB], f16)
                nc.vector.tensor_copy(blo[0:1, :], blo32[0:1, :])
                nc.sync.dma_start(rm[W:W + 1, :], bhi[0:1, :])
                nc.sync.dma_start(rm[W + 1:W + 2, :], blo[0:1, :])

                xm = big.tile([KDIM, T], f16)
                nc.vector.memset(xm[W:W + 2, :], 1.0)
                xc = big.tile([2 * W, T], f16)

            # prenorm sums + x-side fp16 split, pipelined in groups on
            # GPSIMD (x*x, partition allreduces) + ACT (copies with free-dim
            # accumulate) + DVE (residual subtract).
            s2slots = big.tile([1, NXG], f32)
            sxslots = big.tile([1, NXG], f32)
            for g in range(NXG):
                sl = slice(g * XG, (g + 1) * XG)
                hg = x2w.tile([W, XG], f32, tag="hg")
                nc.gpsimd.tensor_mul(hg[:], xbig[:, sl], xbig[:, sl])
                x2g = x2w.tile([W, XG], f32, tag="x2g")
                nc.gpsimd.partition_all_reduce(x2g[:], hg[:], W,
                                               bass_isa.ReduceOp.add)
                xsg = x2w.tile([W, XG], f32, tag="xsg")
                nc.gpsimd.partition_all_reduce(xsg[:], xbig[:, sl], W,
                                               bass_isa.ReduceOp.add)
                xdum = x2w.tile([1, XG], f32, tag="xdum")
                nc.scalar.activation(
                    xdum[:], x2g[0:1, :],
                    mybir.ActivationFunctionType.Copy, scale=-0.5,
                    accum_out=s2slots[0:1, g:g + 1])
                xdum2 = x2w.tile([1, XG], f32, tag="xdum2")
                nc.scalar.activation(
                    xdum2[:], xsg[0:1, :],
                    mybir.ActivationFunctionType.Copy, scale=1.0,
                    accum_out=sxslots[0:1, g:g + 1])
                if not fp32_mm:
                    nc.scalar.copy(xm[0:W, sl], xbig[:, sl])
                    nc.scalar.copy(xc[0:W, sl], xbig[:, sl])
                    xl32g = x2w.tile([W, XG], f32, tag="xl32g")
                    nc.vector.tensor_sub(xl32g[:], xbig[:, sl], xm[0:W, sl])
                    nc.scalar.copy(xc[W:2 * W, sl], xl32g[:])

            # running accumulator for max-scores
            accm = big.tile([TILE, 1], f32)
            nc.vector.memset(accm[:], 0.0)

            labs8 = big.tile([TILE, 8 * NTILES], u32)
            if "idx" not in parts or "smalls" not in parts:
                nc.vector.memset(labs8[:], 0)

            # ---- main loop over token tiles ----
            loop_cm = tc.For_i(0, repeat, 1) if repeat > 1 else contextlib.nullcontext()
            with loop_cm:
              for i in range(NTILES):
                tsl = slice(i * TILE, (i + 1) * TILE)
                pst = ps_s.tile([TILE, KB], f32, tag="scores")
                for c in range(4):
                    csl = slice(c * CH, (c + 1) * CH)
                    if fp32_mm:
                        nc.tensor.matmul(pst[:, csl], xbig[:, tsl],
                                         r_t[:, csl], start=True, stop=True)
                    else:
                        nc.tensor.matmul(pst[:, csl], xm[:, tsl], rm[:, csl],
                                         start=True, stop=False)
                        nc.tensor.matmul(pst[:, csl], xc[:, tsl], rc[:, csl],
                                         start=False, stop=True)

                m8 = work.tile([TILE, 8], f32, tag="m8")
                i8 = labs8[:, i * 8:(i + 1) * 8]
                if "max" in parts:
                    nc.vector.max(m8[:], pst[:])
                else:
                    nc.vector.tensor_copy(m8[:], pst[:, 0:8])
                if "idx" in parts:
                    nc.vector.max_index(i8, m8[:], pst[:])

                ofs = work.tile([TILE, 1], u32, tag="ofs")
                if "smalls" in parts:
                    nc.scalar.add(accm[:], accm[:], m8[:, 0:1])
                    nc.vector.tensor_copy(ofs[:], i8[:, 0:1])
                else:
                    nc.vector.memset(ofs[:], 1)

                if "gat" in parts:
                    gat = gatp.tile([TILE, W], f32, tag="gat")
                    nc.gpsimd.indirect_dma_start(
                        out=gat[:], out_offset=None, in_=ktab_d[:],
                        in_offset=bass.IndirectOffsetOnAxis(ap=ofs[:, 0:1], axis=0))
                    nc.sync.dma_start(xd_d[tsl, :], gat[:])

            # ---- partials ----
            par = big.tile([TILE, 4], f32)
            nc.vector.memset(par[:], 0.0)
            nc.vector.tensor_copy(par[:, 0:1], accm[:])
            nc.vector.tensor_copy(par[0:1, 1:2].broadcast_to([1, NXG]) if False
                                  else par[0:1, 1:2], s2slots[0:1, 0:1])
            nc.sync.dma_start(par_d[:], par[:])
            nc.sync.dma_start(par2_d[0:1, 0:NXG], s2slots[0:1, :])
            nc.sync.dma_start(par2_d[0:1, NXG:2 * NXG], sxslots[0:1, :])
            nc.sync.dma_start(lab_d[:], labs8[:, 0:8 * NTILES:8])

    nc.compile()
    return nc


def _get_nc():
    if "nc" not in _CACHE:
        _CACHE["nc"] = _build()
    return _CACHE["nc"]


def kernel(x: np.ndarray, k: np.ndarray, _trace: bool = False):
    from concourse.bass_utils import run_bass_kernel_spmd

    x = np.ascontiguousarray(np.asarray(x), dtype=np.float32)
    k = np.ascontiguousarray(np.asarray(k), dtype=np.float32)
    kt = np.ascontiguousarray(k.T)

    nc = _get_nc()
    in_maps = [{"xw": x[c], "kt": kt, "ktab": k} for c in range(N)]
    br = run_bass_kernel_spmd(nc, in_maps, core_ids=list(range(N)),
                              trace=_trace)
    results = br.results

    x_l = np.empty((N, T), np.int32)
    x_d = np.empty((N, W, T), np.float32)
    fit_num = 0.0
    sum_x = 0.0
    sum_x2 = 0.0
    for c in range(N):
        r = results[c]
        lab = r["lab"]                       # [128, NTILES] uint32
        x_l[c] = lab.T.reshape(-1).astype(np.int32)
        x_d[c] = r["xd"].T                   # [T, W] -> [W, T]
        par = r["par"].astype(np.float64)
        par2 = r["par2"].astype(np.float64)
        core_sumsq = -2.0 * par2[0, 0:NXG].sum()
        # min dist per token = |x_t|^2 - 2 max_j s;  sum over core's tokens:
        fit_num += core_sumsq - 2.0 * par[:, 0].sum()
        sum_x2 += core_sumsq
        sum_x += par2[0, NXG:2 * NXG].sum()

    size = float(NT * W)
    fit = np.float32(fit_num / NT)
    commit_loss = np.float32(fit_num / NT / W)
    mean = sum_x / size
    prenorm = np.float32(np.sqrt(max(sum_x2 / size - mean * mean, 0.0)))

    if _trace:
        kernel._last_exec_ns = br.exec_time_ns
        kernel._last_mean_exec_ns = br.mean_exec_time_ns
        kernel._last_trace = br.instructions_and_trace
    return x_l, x_d, commit_loss, fit, prenorm
